# revision 11
# baseline (speedup 1.0000x reference)
"""Trainium2 Bass kernel for nn_Attention_53077205844237 (GNN edge softmax).

Computation (reference):
    q   = x_j + e_ij                          # [E, 128]
    w   = tanh(concat([q, x_i], -1) @ W + b)  # [E, 8]
    out = segment_softmax(w, e_row)           # [E, 8], segments = rows

Problem structure (hardcoded): E = 131072 edges, IN = 128, F = 8,
N = 4096 nodes, and e_row = repeat(arange(4096), 32) -- every segment is a
contiguous, 32-edge block.  Since |tanh| < 1, exp() cannot overflow and the
segment-max subtraction is mathematically a no-op -- only a segment *sum*
is needed.  Edges split contiguously across 8 NeuronCores (16384 = 512
whole segments per core): softmax fully local, no collectives.

Fast path ("fp8"): inputs are quantized host-side to fp8 e3m4 (1.8% RMS
element error; the problem tolerance is 2e-2 relative overall) with error
feedback pairing x_j/e_ij: e_ij is quantized AFTER absorbing x_j's
quantization error, so q = x_j + e_ij carries only a single quantization
error.  Per 2048-edge load: DVE adds q = xj + eij (fp8 -> bf16); the PE
runs 2 matmuls per 512-edge chunk (bf16 stationary x bf16/fp8 moving) into
ONE PSUM bank with the 4 chunks partition-stacked at bases {0,32,64,96}
via matmul tile_position, so ACT (tanh, exp) and DVE (32-wide segment sum,
reciprocal, broadcast mul) each run one instruction per bank across 128
partitions instead of 8.  Output is stored bank-stacked ([128, 512] incl.
24 garbage partitions per 32-group) and unshuffled on host.

Fallback ("raw"): the previous all-f32r 3-matmul pipeline (~105 us).
Final fallback: numpy (correct for arbitrary e_row).
"""

import sys
import types
from contextlib import ExitStack

if "/opt/trn_rl_repo" not in sys.path:
    sys.path.insert(0, "/opt/trn_rl_repo")

import numpy as np

# ---------------------------------------------------------------------------
# Optional NTFF-profile hook (used only when _run(trace=True); harmless else).
# ---------------------------------------------------------------------------
if "antenv.axon_hooks" not in sys.modules:
    _hooks_mod = types.ModuleType("antenv.axon_hooks")
    _hook_box = [None]
    _hooks_mod.set_axon_ntff_profile_hook = lambda h: _hook_box.__setitem__(0, h)
    _hooks_mod.get_axon_ntff_profile_hook = lambda: _hook_box[0]
    sys.modules["antenv.axon_hooks"] = _hooks_mod
    try:
        from trn_agent_boot.trn_boot import _ntff_profile_via_ctypes

        _hooks_mod.set_axon_ntff_profile_hook(
            _ntff_profile_via_ctypes("/opt/axon/libaxon_pjrt.so")
        )
    except Exception:
        pass

# Problem constants (hardcoded per the task contract).
E = 131072
IN = 128
F = 8
N_NODES = 4096
DEG = 32
N_CORES = 8
ES = E // N_CORES          # edges per core = 16384

# fp8 pipeline geometry
LD = 2048                  # edges per load == edges per PSUM bank
NB = ES // LD              # loads (= banks) per core = 8
CH = 512                   # psum chunk (col-tile) width
NCT = LD // CH             # chunks per bank = 4
NIN = 4                    # input ring slots
NQ = 3                     # q ring slots
NEW = 4                    # ew ring slots
NWK = 3                    # out ring slots

# Bank plan: tapered tail (last two banks 1024 edges, 256-wide chunks) so
# the serial post-last-load chain (tanh/exp/reduce/recip/mul/store) halves.
BANK_SIZES = [2048] * 7 + [1024, 1024]
NBK = len(BANK_SIZES)
BANK_CHW = [s // NCT for s in BANK_SIZES]
BANK_POS = [sum(BANK_SIZES[:i]) for i in range(NBK)]
BANK_OFF = [sum(BANK_CHW[:i]) for i in range(NBK)]
OUT_COLS = sum(BANK_CHW)

SAFE_INTRA = True          # same-engine RAW sem waits (walrus emits DRAINs)

_COMPILED = {}             # mode -> compiled bass module


def _build_fp8():
    import concourse.bacc as bacc
    from concourse import mybir

    f32 = mybir.dt.float32
    bf16 = mybir.dt.bfloat16
    f8 = mybir.dt.float8e3
    AF = mybir.ActivationFunctionType

    nc = bacc.Bacc("TRN2", target_bir_lowering=False, debug=False,
                   num_devices=N_CORES)

    # Packed input, one fully contiguous DRAM block per load:
    # block b = [xj | eij | xi] (size columns each), row-major [IN, 3*size].
    pk = nc.dram_tensor("pk", [3 * ES * IN], f8, kind="ExternalInput")
    w1 = nc.dram_tensor("W1", [IN, F], bf16, kind="ExternalInput")
    w2 = nc.dram_tensor("W2", [IN, F], bf16, kind="ExternalInput")
    bv = nc.dram_tensor("b", [IN, 1], f32, kind="ExternalInput")  # tiled bias
    # Stacked output: out2[32*c + f, OFF[b] + e] = out[edge POS[b] + chw*c + e, f]
    # Partitions 8..31 of each 32-group are garbage (never read by host).
    out2 = nc.dram_tensor("out2", [IN, OUT_COLS], bf16, kind="ExternalOutput")

    # Loads round-robin over three DMA queues: SP (b%3==0), ACT (1), GP (2).
    # Pipeline-counter semaphores (computed thresholds):
    #   s_mm : +1 per closing matmul of each col-tile (NCT per bank) -- PE
    #          tiles can complete out of order, so each tile signals itself
    #   s_act: +1 per ACT activation   tanh[bb] -> 2bb+1, exp[j] -> 2j+2
    #   s_dve: +1 per DVE op           red[bb] -> 3bb+1, rcp[bb] -> 3bb+2,
    #                                  mul[j] -> 3j+3
    with ExitStack() as ctx:
        def mksem(name):
            return ctx.enter_context(nc.semaphore(name))

        s_in = [mksem("s_in_sp"), mksem("s_in_act"), mksem("s_in_gp")]
        s_out = mksem("s_out")
        s_mm = mksem("s_mm")
        s_act = mksem("s_act")
        s_dve = mksem("s_dve")
        s_const = mksem("s_const")

        in_t = [ctx.enter_context(nc.sbuf_tensor(f"in{r}", [IN, 3 * LD], f8))
                for r in range(NIN)]
        w_t = [ctx.enter_context(nc.sbuf_tensor(f"w{r}", [IN, CH], f32))
               for r in range(2)]
        ew_t = [ctx.enter_context(nc.sbuf_tensor(f"ew{r}", [IN, CH], bf16))
                for r in range(NEW)]
        o_t = [ctx.enter_context(nc.sbuf_tensor(f"o{r}", [IN, CH], bf16))
               for r in range(NWK)]
        dn_t = [ctx.enter_context(nc.sbuf_tensor(f"dn{r}", [IN, CH // DEG],
                                                 bf16)) for r in range(2)]
        rc_t = [ctx.enter_context(nc.sbuf_tensor(f"rc{r}", [IN, CH // DEG],
                                                 bf16)) for r in range(2)]
        ps_t = [ctx.enter_context(nc.psum_tensor(f"ps{r}", [IN, CH], f32))
                for r in range(8)]
        w1_t = ctx.enter_context(nc.sbuf_tensor("w1s", [IN, F], bf16))
        w2_t = ctx.enter_context(nc.sbuf_tensor("w2s", [IN, F], bf16))
        b_t = ctx.enter_context(nc.sbuf_tensor("bs", [IN, 1], f32))

        def pk_block(b):
            off = 3 * IN * BANK_POS[b]
            n = 3 * IN * BANK_SIZES[b]
            return pk[off:off + n].rearrange("(p c) -> p c", p=IN)

        def load_issue(eng, qi, b):
            # slot-reuse: wait until PE finished the bank that last used it
            if b >= NIN:
                eng.wait_ge(s_mm, NCT * (b - NIN + 1))
            eng.dma_start(out=in_t[b % NIN][:, 0:3 * BANK_SIZES[b]],
                          in_=pk_block(b)).then_inc(s_in[qi], 16)

        with nc.Block() as block:

            @block.sync
            def _(sp):
                for b in (0, 3, 6):
                    load_issue(sp, 0, b)

            @block.scalar
            def _(act):
                def act_exp(j):
                    # exp of bank j, one bank behind tanh: the gap gives the
                    # producer writes time to land after their sem fires
                    cw = BANK_CHW[j]
                    if SAFE_INTRA:
                        act.wait_ge(s_act, 2 * j + 1)
                    if j >= NEW:
                        act.wait_ge(s_dve, 3 * (j - NEW) + 3)
                    act.activation(
                        out=ew_t[j % NEW][:, 0:cw], in_=w_t[j % 2][:, 0:cw],
                        func=AF.Exp,
                    ).then_inc(s_act, 1)

                def act_bank(bb):
                    cw = BANK_CHW[bb]
                    act.wait_ge(s_mm, NCT * (bb + 1))
                    if bb >= 1:
                        act_exp(bb - 1)
                    act.activation(
                        out=w_t[bb % 2][:, 0:cw],
                        in_=ps_t[bb % 8][:, 0:cw],
                        func=AF.Tanh, bias=b_t[:, 0:1],
                    ).then_inc(s_act, 1)

                load_issue(act, 1, 1)
                act.wait_ge(s_const, 48)
                act_bank(0)
                act_bank(1)
                load_issue(act, 1, 4)
                act_bank(2)
                act_bank(3)
                load_issue(act, 1, 7)
                for bb in range(4, NBK):
                    act_bank(bb)
                act_exp(NBK - 1)

            @block.tensor
            def _(pe):
                pe.wait_ge(s_const, 32)
                for b in range(NBK):
                    size = BANK_SIZES[b]
                    cw = BANK_CHW[b]
                    pe.wait_ge(s_in[b % 3], 16 * (b // 3 + 1))
                    if b >= 8:
                        # psum bank reuse: tanh of bank b-8 must be done
                        pe.wait_ge(s_act, 2 * (b - 8) + 1)
                    ps = ps_t[b % 8]
                    it = in_t[b % NIN]
                    for c in range(NCT):
                        po = ps[32 * c:32 * c + F, 0:cw]
                        csl = slice(c * cw, (c + 1) * cw)
                        esl = slice(size + c * cw, size + (c + 1) * cw)
                        isl = slice(2 * size + c * cw, 2 * size + (c + 1) * cw)
                        pe.matmul(po, w1_t[:], it[:, csl],
                                  start=True, stop=False,
                                  tile_position=(0, 32 * c))
                        pe.matmul(po, w1_t[:], it[:, esl],
                                  start=False, stop=False,
                                  tile_position=(0, 32 * c))
                        # inc per col-tile: matmuls on different PE tiles can
                        # complete out of order, so the bank is only ready
                        # once ALL four closing matmuls have signalled
                        pe.matmul(po, w2_t[:], it[:, isl],
                                  start=False, stop=True,
                                  tile_position=(0, 32 * c)
                                  ).then_inc(s_mm, 1)

            @block.vector
            def _(dve):
                def dve_mul(j):
                    # mul of bank j, one bank behind reduce/recip: the gap
                    # gives producer writes time to land after their sem fires
                    cw = BANK_CHW[j]
                    if SAFE_INTRA:
                        dve.wait_ge(s_dve, 3 * j + 2)
                    if j >= NWK:
                        dve.wait_ge(s_out, 16 * (j - NWK + 1))
                    dve.tensor_mul(
                        out=o_t[j % NWK][:, 0:cw].rearrange(
                            "p (n d) -> p n d", d=DEG),
                        in0=ew_t[j % NEW][:, 0:cw].rearrange(
                            "p (n d) -> p n d", d=DEG),
                        in1=rc_t[j % 2][:, 0:cw // DEG].unsqueeze(
                            -1).broadcast_to([IN, cw // DEG, DEG]),
                    ).then_inc(s_dve, 1)

                for bb in range(NBK):
                    cw = BANK_CHW[bb]
                    dve.wait_ge(s_act, 2 * bb + 2)
                    if bb >= 1:
                        dve_mul(bb - 1)
                    with nc.allow_low_precision(
                            "32-wide segment sum of exp(tanh) in bf16; "
                            "error budget allows ~0.4%"):
                        dve.reduce_sum(
                            out=dn_t[bb % 2][:, 0:cw // DEG],
                            in_=ew_t[bb % NEW][:, 0:cw].rearrange(
                                "p (n d) -> p n d", d=DEG),
                            axis=mybir.AxisListType.X,
                        ).then_inc(s_dve, 1)
                        if SAFE_INTRA:
                            dve.wait_ge(s_dve, 3 * bb + 1)
                        dve.reciprocal(out=rc_t[bb % 2][:, 0:cw // DEG],
                                       in_=dn_t[bb % 2][:, 0:cw // DEG]
                                       ).then_inc(s_dve, 1)
                dve_mul(NBK - 1)

            @block.gpsimd
            def _(gp):
                gp.dma_start(out=w1_t[:], in_=w1[:]).then_inc(s_const, 16)
                gp.dma_start(out=w2_t[:], in_=w2[:]).then_inc(s_const, 16)
                gp.dma_start(out=b_t[:], in_=bv[:]).then_inc(s_const, 16)
                load_issue(gp, 2, 2)
                load_issue(gp, 2, 5)

                def store(b):
                    cw = BANK_CHW[b]
                    gp.wait_ge(s_dve, 3 * b + 3)
                    gp.dma_start(
                        out=out2[:, BANK_OFF[b]:BANK_OFF[b] + cw],
                        in_=o_t[b % NWK][:, 0:cw]).then_inc(s_out, 16)

                for b in range(3):
                    store(b)
                load_issue(gp, 2, 8)
                for b in range(3, NBK):
                    store(b)
                gp.wait_ge(s_out, 16 * NBK)

    nc.compile()
    return nc


# ---------------------------------------------------------------------------
# Fallback: previous all-f32r raw pipeline (measured ~105 us end-to-end).
# ---------------------------------------------------------------------------
LDR = 2048                 # raw-path input DMA batch (edges)
CHR = 512                  # raw-path matmul chunk


def _load_plan_raw():
    tail = [CHR, CHR, CHR, CHR // 2, CHR // 4, CHR // 4]
    loads = []
    pos = 0
    while pos < ES - sum(tail):
        loads.append((pos, LDR))
        pos += LDR
    for sz in tail:
        loads.append((pos, sz))
        pos += sz
    assert pos == ES, (pos, ES)
    return loads


def _build_bass_raw():
    import concourse.bacc as bacc
    from concourse import mybir

    f32 = mybir.dt.float32
    f32r = mybir.dt.float32r
    AF = mybir.ActivationFunctionType

    nc = bacc.Bacc("TRN2", target_bir_lowering=False, debug=False,
                   num_devices=N_CORES)

    xjT = nc.dram_tensor("xjT", [IN, ES], f32r, kind="ExternalInput")
    eijT = nc.dram_tensor("eijT", [IN, ES], f32r, kind="ExternalInput")
    xiT = nc.dram_tensor("xiT", [IN, ES], f32r, kind="ExternalInput")
    w1 = nc.dram_tensor("W1", [IN, F], f32r, kind="ExternalInput")
    w2 = nc.dram_tensor("W2", [IN, F], f32r, kind="ExternalInput")
    bv = nc.dram_tensor("b", [F, 1], f32, kind="ExternalInput")
    outT = nc.dram_tensor("outT", [F, ES], f32, kind="ExternalOutput")

    loads = _load_plan_raw()
    NB_ = len(loads)
    NIN_ = 5
    NWK_ = 3
    NEW_ = NWK_

    with ExitStack() as ctx:
        def mksem(name):
            return ctx.enter_context(nc.semaphore(name))

        s_xj = [mksem(f"s_xj{r}") for r in range(NIN_)]
        s_eij = [mksem(f"s_eij{r}") for r in range(NIN_)]
        s_xi = [mksem(f"s_xi{r}") for r in range(NIN_)]
        s_out = [mksem(f"s_out{r}") for r in range(NWK_)]
        s_mm = mksem("s_mm")
        s_red = mksem("s_red")
        s_rcp = mksem("s_rcp")
        s_psf = mksem("s_psf")
        s_exp = mksem("s_exp")
        s_mul = mksem("s_mul")
        s_const = mksem("s_const")

        in_xj = [ctx.enter_context(nc.sbuf_tensor(f"in_xj{r}", [IN, LDR], f32r))
                 for r in range(NIN_)]
        in_eij = [ctx.enter_context(nc.sbuf_tensor(f"in_eij{r}", [IN, LDR], f32r))
                  for r in range(NIN_)]
        in_xi = [ctx.enter_context(nc.sbuf_tensor(f"in_xi{r}", [IN, LDR], f32r))
                 for r in range(NIN_)]
        w_t = [ctx.enter_context(nc.sbuf_tensor(f"w{r}", [F, LDR], f32))
               for r in range(NWK_)]
        ew_t = [ctx.enter_context(nc.sbuf_tensor(f"ew{r}", [F, LDR], f32))
                for r in range(NEW_)]
        o_t = [ctx.enter_context(nc.sbuf_tensor(f"o{r}", [F, LDR], f32))
               for r in range(NWK_)]
        dn_t = ctx.enter_context(nc.sbuf_tensor("dn", [F, LDR // DEG], f32))
        rc_t = ctx.enter_context(nc.sbuf_tensor("rc", [F, LDR // DEG], f32))
        ps_t = [ctx.enter_context(nc.psum_tensor(f"ps{r}", [F, LDR], f32))
                for r in range(2)]
        w1_t = ctx.enter_context(nc.sbuf_tensor("w1s", [IN, F], f32r))
        w2_t = ctx.enter_context(nc.sbuf_tensor("w2s", [IN, F], f32r))
        b_t = ctx.enter_context(nc.sbuf_tensor("bs", [F, 1], f32))

        with nc.Block() as block:

            @block.sync
            def _(sp):
                for b, (pos, size) in enumerate(loads):
                    sl = slice(pos, pos + size)
                    if b >= NIN_:
                        sp.wait_ge(s_mm, b - (NIN_ - 1))
                    sp.dma_start(out=in_xj[b % NIN_][:, 0:size],
                                 in_=xjT[:, sl]).then_inc(s_xj[b % NIN_], 16)
                    if b % 2 == 0:
                        sp.dma_start(out=in_xi[b % NIN_][:, 0:size],
                                     in_=xiT[:, sl]).then_inc(s_xi[b % NIN_], 16)

            @block.scalar
            def _(act):
                def act_tail(bb):
                    bsz = loads[bb][1]
                    act.wait_ge(s_mm, bb + 1)
                    if bb >= NEW_:
                        act.wait_ge(s_mul, bb - (NEW_ - 1))
                    act.activation(
                        out=w_t[bb % NWK_][:, 0:bsz],
                        in_=ps_t[bb % 2][:, 0:bsz],
                        func=AF.Tanh, bias=b_t[:, 0:1],
                    ).then_inc(s_psf, 1)
                    if SAFE_INTRA:
                        act.wait_ge(s_psf, bb + 1)
                    act.activation(
                        out=ew_t[bb % NEW_][:, 0:bsz],
                        in_=w_t[bb % NWK_][:, 0:bsz],
                        func=AF.Exp,
                    ).then_inc(s_exp, 1)

                for b, (pos, size) in enumerate(loads):
                    sl = slice(pos, pos + size)
                    if b >= NIN_:
                        act.wait_ge(s_mm, b - (NIN_ - 1))
                    act.dma_start(out=in_eij[b % NIN_][:, 0:size],
                                  in_=eijT[:, sl]).then_inc(s_eij[b % NIN_], 16)
                    if b % 2 == 1:
                        act.dma_start(out=in_xi[b % NIN_][:, 0:size],
                                      in_=xiT[:, sl]).then_inc(s_xi[b % NIN_], 16)
                    if b >= 2:
                        bb = b - 2
                        if bb == 0:
                            act.wait_ge(s_const, 48)
                        act_tail(bb)
                for bb in (NB_ - 2, NB_ - 1):
                    act_tail(bb)

            @block.tensor
            def _(pe):
                pe.wait_ge(s_const, 48)
                for b, (pos, size) in enumerate(loads):
                    r = b % NIN_
                    n_use = b // NIN_ + 1
                    pe.wait_ge(s_xj[r], 16 * n_use)
                    pe.wait_ge(s_eij[r], 16 * n_use)
                    pe.wait_ge(s_xi[r], 16 * n_use)
                    if b >= 2:
                        pe.wait_ge(s_psf, b - 1)
                    ps = ps_t[b % 2]
                    nch = (size + CHR - 1) // CHR
                    for c in range(nch):
                        cw = min(CHR, size - c * CHR)
                        csl = slice(c * CHR, c * CHR + cw)
                        pe.matmul(ps[:, csl],
                                  w1_t[:], in_xj[b % NIN_][:, csl],
                                  start=True, stop=False)
                        pe.matmul(ps[:, csl],
                                  w1_t[:], in_eij[b % NIN_][:, csl],
                                  start=False, stop=False)
                        last = pe.matmul(ps[:, csl],
                                         w2_t[:], in_xi[b % NIN_][:, csl],
                                         start=False, stop=True)
                    last.then_inc(s_mm, 1)

            @block.vector
            def _(dve):
                for b, (pos, size) in enumerate(loads):
                    nseg = size // DEG
                    dve.wait_ge(s_exp, b + 1)
                    ew = ew_t[b % NEW_]
                    dve.reduce_sum(
                        out=dn_t[:, 0:nseg],
                        in_=ew[:, 0:size].rearrange("p (n d) -> p n d", d=DEG),
                        axis=mybir.AxisListType.X,
                    ).then_inc(s_red, 1)
                    if SAFE_INTRA:
                        dve.wait_ge(s_red, b + 1)
                    dve.reciprocal(
                        out=rc_t[:, 0:nseg], in_=dn_t[:, 0:nseg]
                    ).then_inc(s_rcp, 1)
                    if SAFE_INTRA:
                        dve.wait_ge(s_rcp, b + 1)
                    if b >= NWK_:
                        dve.wait_ge(s_out[b % NWK_], 16 * ((b - NWK_) // NWK_ + 1))
                    dve.tensor_mul(
                        out=o_t[b % NWK_][:, 0:size].rearrange(
                            "p (n d) -> p n d", d=DEG),
                        in0=ew[:, 0:size].rearrange("p (n d) -> p n d", d=DEG),
                        in1=rc_t[:, 0:nseg].unsqueeze(-1).broadcast_to(
                            [F, nseg, DEG]),
                    ).then_inc(s_mul, 1)

            @block.gpsimd
            def _(gp):
                gp.dma_start(out=w1_t[:], in_=w1[:]).then_inc(s_const, 16)
                gp.dma_start(out=w2_t[:], in_=w2[:]).then_inc(s_const, 16)
                gp.dma_start(out=b_t[:], in_=bv[:]).then_inc(s_const, 16)
                for b, (pos, size) in enumerate(loads):
                    sl = slice(pos, pos + size)
                    gp.wait_ge(s_mul, b + 1)
                    gp.dma_start(
                        out=outT[:, sl],
                        in_=o_t[b % NWK_][:, 0:size],
                    ).then_inc(s_out[b % NWK_], 16)
                for r in range(NWK_):
                    n_r = len(range(r, NB_, NWK_))
                    gp.wait_ge(s_out[r], 16 * n_r)

    nc.compile()
    return nc


def _get_compiled(mode):
    if mode not in _COMPILED:
        _COMPILED[mode] = _build_fp8() if mode == "fp8" else _build_bass_raw()
    return _COMPILED[mode]


def _prep_inputs_fp8(x_i, x_j, e_ij, W, b):
    import ml_dtypes

    F8 = ml_dtypes.float8_e3m4
    BF16 = ml_dtypes.bfloat16

    W = np.asarray(W, dtype=np.float32)
    W1 = np.ascontiguousarray(W[:IN]).astype(BF16)
    W2 = np.ascontiguousarray(W[IN:]).astype(BF16)
    bias = np.asarray(b, dtype=np.float32).reshape(F)
    btile = np.zeros((IN, 1), np.float32)
    for cc in range(NCT):
        btile[32 * cc:32 * cc + F, 0] = bias

    in_maps = []
    for c in range(N_CORES):
        sl = slice(c * ES, (c + 1) * ES)
        xjT = np.ascontiguousarray(np.asarray(x_j[sl], np.float32).T)
        eijT = np.ascontiguousarray(np.asarray(e_ij[sl], np.float32).T)
        xiT = np.ascontiguousarray(np.asarray(x_i[sl], np.float32).T)
        xj8 = xjT.astype(F8)
        # error feedback: fold xj's quantization error into eij before its
        # quantization, so q = xj + eij carries a single quantization error
        eij8 = (eijT + (xjT - xj8.astype(np.float32))).astype(F8)
        xi8 = xiT.astype(F8)
        # pack per load: contiguous [IN, 3*size] block b = [xj | eij | xi]
        blocks = []
        for b in range(NBK):
            pos, size = BANK_POS[b], BANK_SIZES[b]
            blk = np.concatenate([xj8[:, pos:pos + size],
                                  eij8[:, pos:pos + size],
                                  xi8[:, pos:pos + size]], axis=1)
            blocks.append(np.ascontiguousarray(blk).reshape(-1))
        pk = np.concatenate(blocks)
        in_maps.append({
            "pk": pk,
            "W1": W1,
            "W2": W2,
            "b": btile,
        })
    return in_maps


def _gather_fp8(res):
    out = np.empty((E, F), dtype=np.float32)
    for c in range(N_CORES):
        o2 = np.asarray(res.results[c]["out2"]).astype(np.float32)
        oc = out[c * ES:(c + 1) * ES]
        for b in range(NBK):
            pos, cw = BANK_POS[b], BANK_CHW[b]
            blk = o2.reshape(NCT, 32, OUT_COLS)[:, :F,
                                                BANK_OFF[b]:BANK_OFF[b] + cw]
            oc[pos:pos + NCT * cw] = blk.transpose(0, 2, 1).reshape(-1, F)
    return out


def _prep_inputs_raw(x_i, x_j, e_ij, W, b):
    W = np.ascontiguousarray(np.asarray(W, dtype=np.float32))
    bias = np.asarray(b, dtype=np.float32).reshape(F, 1)
    W1 = np.ascontiguousarray(W[:IN])
    W2 = np.ascontiguousarray(W[IN:])
    in_maps = []
    for c in range(N_CORES):
        sl = slice(c * ES, (c + 1) * ES)
        in_maps.append({
            "xjT": np.ascontiguousarray(np.asarray(x_j[sl]).T),
            "eijT": np.ascontiguousarray(np.asarray(e_ij[sl]).T),
            "xiT": np.ascontiguousarray(np.asarray(x_i[sl]).T),
            "W1": W1,
            "W2": W2,
            "b": bias,
        })
    return in_maps


def _gather_raw(res):
    out = np.empty((E, F), dtype=np.float32)
    for c in range(N_CORES):
        out[c * ES:(c + 1) * ES] = np.asarray(res.results[c]["outT"]).T
    return out


def _run_device(x_i, x_j, e_ij, W, b, trace=False, tmpdir=None,
                trace_cores=None, mode="fp8"):
    from concourse.bass_utils import run_bass_kernel_spmd

    nc = _get_compiled(mode)
    if mode == "fp8":
        in_maps = _prep_inputs_fp8(x_i, x_j, e_ij, W, b)
    else:
        in_maps = _prep_inputs_raw(x_i, x_j, e_ij, W, b)

    kwargs = {}
    if trace:
        kwargs.update(trace=True,
                      trace_cores=(trace_cores if trace_cores is not None
                                   else list(range(N_CORES))),
                      tmpdir=tmpdir)
    res = run_bass_kernel_spmd(nc, in_maps, core_ids=list(range(N_CORES)),
                               **kwargs)

    out = _gather_fp8(res) if mode == "fp8" else _gather_raw(res)
    return out, res


def _numpy_fallback(x_i, x_j, e_ij, adj, e_row, W, b):
    """Correct for arbitrary e_row (matches the reference semantics)."""
    x_i = np.asarray(x_i, np.float32)
    x_j = np.asarray(x_j, np.float32)
    e_ij = np.asarray(e_ij, np.float32)
    W = np.asarray(W, np.float32)
    b = np.asarray(b, np.float32)
    e_row = np.asarray(e_row).astype(np.int64)
    n = np.asarray(adj).shape[0]
    q = x_j + e_ij
    z = q @ W[:q.shape[1]] + x_i @ W[q.shape[1]:] + b
    w = np.tanh(z)
    m = np.full((n, w.shape[1]), -9e15, np.float32)
    np.maximum.at(m, e_row, w)
    ew = np.exp(w - m[e_row])
    denom = np.zeros((n, w.shape[1]), np.float32)
    np.add.at(denom, e_row, ew)
    return (ew / denom[e_row]).astype(np.float32)


def _is_fast_path(x_i, x_j, e_ij, adj, e_row, W, b):
    try:
        if np.asarray(x_i).shape != (E, IN):
            return False
        if np.asarray(x_j).shape != (E, IN):
            return False
        if np.asarray(e_ij).shape != (E, IN):
            return False
        if np.asarray(W).shape != (2 * IN, F):
            return False
        if np.asarray(b).reshape(-1).shape != (F,):
            return False
        if np.asarray(adj).shape[0] != N_NODES:
            return False
        er = np.asarray(e_row).reshape(-1)
        if er.shape != (E,):
            return False
        expected = np.repeat(np.arange(N_NODES, dtype=np.int64), DEG)
        return bool(np.array_equal(er.astype(np.int64), expected))
    except Exception:
        return False


def kernel(x_i, x_j, e_ij, adj, e_row, e_col, W, b, **_unused):
    if _is_fast_path(x_i, x_j, e_ij, adj, e_row, W, b):
        for mode in ("fp8", "raw"):
            try:
                out, _ = _run_device(x_i, x_j, e_ij, W, b, mode=mode)
                return out
            except Exception as e:  # fail safe: correct > fast
                print(f"kernel: device path '{mode}' failed "
                      f"({type(e).__name__}: {e}); trying next",
                      file=sys.stderr)
    return _numpy_fallback(x_i, x_j, e_ij, adj, e_row, W, b)


# revision 12
# speedup vs baseline: 1.0196x; 1.0196x over previous
"""Trainium2 Bass kernel for nn_Attention_53077205844237 (GNN edge softmax).

Computation (reference):
    q   = x_j + e_ij                          # [E, 128]
    w   = tanh(concat([q, x_i], -1) @ W + b)  # [E, 8]
    out = segment_softmax(w, e_row)           # [E, 8], segments = rows

Problem structure (hardcoded): E = 131072 edges, IN = 128, F = 8,
N = 4096 nodes, and e_row = repeat(arange(4096), 32) -- every segment is a
contiguous, 32-edge block.  Since |tanh| < 1, exp() cannot overflow and the
segment-max subtraction is mathematically a no-op -- only a segment *sum*
is needed.  Edges split contiguously across 8 NeuronCores (16384 = 512
whole segments per core): softmax fully local, no collectives.

Fast path ("fp8"): inputs are quantized host-side to fp8 e3m4 (1.8% RMS
element error; the problem tolerance is 2e-2 relative overall) with error
feedback pairing x_j/e_ij: e_ij is quantized AFTER absorbing x_j's
quantization error, so q = x_j + e_ij carries only a single quantization
error.  Per 2048-edge load: DVE adds q = xj + eij (fp8 -> bf16); the PE
runs 2 matmuls per 512-edge chunk (bf16 stationary x bf16/fp8 moving) into
ONE PSUM bank with the 4 chunks partition-stacked at bases {0,32,64,96}
via matmul tile_position, so ACT (tanh, exp) and DVE (32-wide segment sum,
reciprocal, broadcast mul) each run one instruction per bank across 128
partitions instead of 8.  Output is stored bank-stacked ([128, 512] incl.
24 garbage partitions per 32-group) and unshuffled on host.

Fallback ("raw"): the previous all-f32r 3-matmul pipeline (~105 us).
Final fallback: numpy (correct for arbitrary e_row).
"""

import sys
import types
from contextlib import ExitStack

if "/opt/trn_rl_repo" not in sys.path:
    sys.path.insert(0, "/opt/trn_rl_repo")

import numpy as np

# ---------------------------------------------------------------------------
# Optional NTFF-profile hook (used only when _run(trace=True); harmless else).
# ---------------------------------------------------------------------------
if "antenv.axon_hooks" not in sys.modules:
    _hooks_mod = types.ModuleType("antenv.axon_hooks")
    _hook_box = [None]
    _hooks_mod.set_axon_ntff_profile_hook = lambda h: _hook_box.__setitem__(0, h)
    _hooks_mod.get_axon_ntff_profile_hook = lambda: _hook_box[0]
    sys.modules["antenv.axon_hooks"] = _hooks_mod
    try:
        from trn_agent_boot.trn_boot import _ntff_profile_via_ctypes

        _hooks_mod.set_axon_ntff_profile_hook(
            _ntff_profile_via_ctypes("/opt/axon/libaxon_pjrt.so")
        )
    except Exception:
        pass

# Problem constants (hardcoded per the task contract).
E = 131072
IN = 128
F = 8
N_NODES = 4096
DEG = 32
N_CORES = 8
ES = E // N_CORES          # edges per core = 16384

# fp8 pipeline geometry
LD = 2048                  # edges per load == edges per PSUM bank
NB = ES // LD              # loads (= banks) per core = 8
CH = 512                   # psum chunk (col-tile) width
NCT = LD // CH             # chunks per bank = 4
NIN = 4                    # input ring slots
NQ = 3                     # q ring slots
NEW = 4                    # ew ring slots
NWK = 3                    # out ring slots

# Bank plan: tapered tail (last two banks 1024 edges, 256-wide chunks) so
# the serial post-last-load chain (tanh/exp/reduce/recip/mul/store) halves.
BANK_SIZES = [2048] * 7 + [1024, 1024]
NBK = len(BANK_SIZES)
BANK_CHW = [s // NCT for s in BANK_SIZES]
BANK_POS = [sum(BANK_SIZES[:i]) for i in range(NBK)]
BANK_OFF = [sum(BANK_CHW[:i]) for i in range(NBK)]
OUT_COLS = sum(BANK_CHW)

SAFE_INTRA = True          # same-engine RAW sem waits (walrus emits DRAINs)

_COMPILED = {}             # mode -> compiled bass module


def _build_fp8():
    import concourse.bacc as bacc
    from concourse import mybir

    f32 = mybir.dt.float32
    bf16 = mybir.dt.bfloat16
    f8 = mybir.dt.float8e3
    AF = mybir.ActivationFunctionType

    nc = bacc.Bacc("TRN2", target_bir_lowering=False, debug=False,
                   num_devices=N_CORES)

    # Packed input, one fully contiguous DRAM block per load:
    # block b = [xj | eij | xi] (size columns each), row-major [IN, 3*size].
    pk = nc.dram_tensor("pk", [3 * ES * IN], f8, kind="ExternalInput")
    w1 = nc.dram_tensor("W1", [IN, F], bf16, kind="ExternalInput")
    w2 = nc.dram_tensor("W2", [IN, F], bf16, kind="ExternalInput")
    bv = nc.dram_tensor("b", [IN, 1], f32, kind="ExternalInput")  # tiled bias
    # Stacked output: out2[32*c + f, OFF[b] + e] = out[edge POS[b] + chw*c + e, f]
    # Partitions 8..31 of each 32-group are garbage (never read by host).
    out2 = nc.dram_tensor("out2", [IN, OUT_COLS], bf16, kind="ExternalOutput")

    # Loads round-robin over three DMA queues: SP (b%3==0), ACT (1), GP (2).
    # Pipeline-counter semaphores (computed thresholds):
    #   s_mm : +1 per closing matmul of each col-tile (NCT per bank) -- PE
    #          tiles can complete out of order, so each tile signals itself
    #   s_act: +1 per ACT activation   tanh[bb] -> 2bb+1, exp[j] -> 2j+2
    #   s_dve: +1 per DVE op           red[bb] -> 3bb+1, rcp[bb] -> 3bb+2,
    #                                  mul[j] -> 3j+3
    with ExitStack() as ctx:
        def mksem(name):
            return ctx.enter_context(nc.semaphore(name))

        s_in = [mksem("s_in_sp"), mksem("s_in_act")]
        s_out = mksem("s_out")
        s_mm = mksem("s_mm")
        s_act = mksem("s_act")
        s_dve = mksem("s_dve")
        s_const = mksem("s_const")

        in_t = [ctx.enter_context(nc.sbuf_tensor(f"in{r}", [IN, 3 * LD], f8))
                for r in range(NIN)]
        w_t = [ctx.enter_context(nc.sbuf_tensor(f"w{r}", [IN, CH], f32))
               for r in range(2)]
        ew_t = [ctx.enter_context(nc.sbuf_tensor(f"ew{r}", [IN, CH], bf16))
                for r in range(NEW)]
        o_t = [ctx.enter_context(nc.sbuf_tensor(f"o{r}", [IN, CH], bf16))
               for r in range(NWK)]
        dn_t = [ctx.enter_context(nc.sbuf_tensor(f"dn{r}", [IN, CH // DEG],
                                                 bf16)) for r in range(2)]
        rc_t = [ctx.enter_context(nc.sbuf_tensor(f"rc{r}", [IN, CH // DEG],
                                                 bf16)) for r in range(2)]
        ps_t = [ctx.enter_context(nc.psum_tensor(f"ps{r}", [IN, CH], f32))
                for r in range(8)]
        w1_t = ctx.enter_context(nc.sbuf_tensor("w1s", [IN, F], bf16))
        w2_t = ctx.enter_context(nc.sbuf_tensor("w2s", [IN, F], bf16))
        b_t = ctx.enter_context(nc.sbuf_tensor("bs", [IN, 1], f32))

        def pk_block(b):
            off = 3 * IN * BANK_POS[b]
            n = 3 * IN * BANK_SIZES[b]
            return pk[off:off + n].rearrange("(p c) -> p c", p=IN)

        def load_issue(eng, qi, b):
            # slot-reuse: wait until PE finished the bank that last used it
            if b >= NIN:
                eng.wait_ge(s_mm, NCT * (b - NIN + 1))
            eng.dma_start(out=in_t[b % NIN][:, 0:3 * BANK_SIZES[b]],
                          in_=pk_block(b)).then_inc(s_in[qi], 16)

        with nc.Block() as block:

            @block.sync
            def _(sp):
                for b in range(0, NBK, 2):
                    load_issue(sp, 0, b)

            @block.scalar
            def _(act):
                def act_exp(j):
                    # exp of bank j, one bank behind tanh: the gap gives the
                    # producer writes time to land after their sem fires
                    cw = BANK_CHW[j]
                    if SAFE_INTRA:
                        act.wait_ge(s_act, 2 * j + 1)
                    if j >= NEW:
                        act.wait_ge(s_dve, 3 * (j - NEW) + 3)
                    act.activation(
                        out=ew_t[j % NEW][:, 0:cw], in_=w_t[j % 2][:, 0:cw],
                        func=AF.Exp,
                    ).then_inc(s_act, 1)

                def act_bank(bb):
                    cw = BANK_CHW[bb]
                    act.wait_ge(s_mm, NCT * (bb + 1))
                    if bb >= 1:
                        act_exp(bb - 1)
                    act.activation(
                        out=w_t[bb % 2][:, 0:cw],
                        in_=ps_t[bb % 8][:, 0:cw],
                        func=AF.Tanh, bias=b_t[:, 0:1],
                    ).then_inc(s_act, 1)

                load_issue(act, 1, 1)
                act.wait_ge(s_const, 48)
                load_issue(act, 1, 3)
                act_bank(0)
                act_bank(1)
                load_issue(act, 1, 5)
                act_bank(2)
                act_bank(3)
                load_issue(act, 1, 7)
                for bb in range(4, NBK):
                    act_bank(bb)
                act_exp(NBK - 1)

            @block.tensor
            def _(pe):
                pe.wait_ge(s_const, 32)
                for b in range(NBK):
                    size = BANK_SIZES[b]
                    cw = BANK_CHW[b]
                    pe.wait_ge(s_in[b % 2], 16 * (b // 2 + 1))
                    if b >= 8:
                        # psum bank reuse: tanh of bank b-8 must be done
                        pe.wait_ge(s_act, 2 * (b - 8) + 1)
                    ps = ps_t[b % 8]
                    it = in_t[b % NIN]
                    for c in range(NCT):
                        po = ps[32 * c:32 * c + F, 0:cw]
                        csl = slice(c * cw, (c + 1) * cw)
                        esl = slice(size + c * cw, size + (c + 1) * cw)
                        isl = slice(2 * size + c * cw, 2 * size + (c + 1) * cw)
                        pe.matmul(po, w1_t[:], it[:, csl],
                                  start=True, stop=False,
                                  tile_position=(0, 32 * c))
                        pe.matmul(po, w1_t[:], it[:, esl],
                                  start=False, stop=False,
                                  tile_position=(0, 32 * c))
                        # inc per col-tile: matmuls on different PE tiles can
                        # complete out of order, so the bank is only ready
                        # once ALL four closing matmuls have signalled
                        pe.matmul(po, w2_t[:], it[:, isl],
                                  start=False, stop=True,
                                  tile_position=(0, 32 * c)
                                  ).then_inc(s_mm, 1)

            @block.vector
            def _(dve):
                def dve_mul(j):
                    # mul of bank j, one bank behind reduce/recip: the gap
                    # gives producer writes time to land after their sem fires
                    cw = BANK_CHW[j]
                    if SAFE_INTRA:
                        dve.wait_ge(s_dve, 3 * j + 2)
                    if j >= NWK:
                        dve.wait_ge(s_out, 16 * (j - NWK + 1))
                    dve.tensor_mul(
                        out=o_t[j % NWK][:, 0:cw].rearrange(
                            "p (n d) -> p n d", d=DEG),
                        in0=ew_t[j % NEW][:, 0:cw].rearrange(
                            "p (n d) -> p n d", d=DEG),
                        in1=rc_t[j % 2][:, 0:cw // DEG].unsqueeze(
                            -1).broadcast_to([IN, cw // DEG, DEG]),
                    ).then_inc(s_dve, 1)

                for bb in range(NBK):
                    cw = BANK_CHW[bb]
                    dve.wait_ge(s_act, 2 * bb + 2)
                    if bb >= 1:
                        dve_mul(bb - 1)
                    with nc.allow_low_precision(
                            "32-wide segment sum of exp(tanh) in bf16; "
                            "error budget allows ~0.4%"):
                        dve.reduce_sum(
                            out=dn_t[bb % 2][:, 0:cw // DEG],
                            in_=ew_t[bb % NEW][:, 0:cw].rearrange(
                                "p (n d) -> p n d", d=DEG),
                            axis=mybir.AxisListType.X,
                        ).then_inc(s_dve, 1)
                        if SAFE_INTRA:
                            dve.wait_ge(s_dve, 3 * bb + 1)
                        dve.reciprocal(out=rc_t[bb % 2][:, 0:cw // DEG],
                                       in_=dn_t[bb % 2][:, 0:cw // DEG]
                                       ).then_inc(s_dve, 1)
                dve_mul(NBK - 1)

            @block.gpsimd
            def _(gp):
                gp.dma_start(out=w1_t[:], in_=w1[:]).then_inc(s_const, 16)
                gp.dma_start(out=w2_t[:], in_=w2[:]).then_inc(s_const, 16)
                gp.dma_start(out=b_t[:], in_=bv[:]).then_inc(s_const, 16)
                def store(b):
                    cw = BANK_CHW[b]
                    gp.wait_ge(s_dve, 3 * b + 3)
                    gp.dma_start(
                        out=out2[:, BANK_OFF[b]:BANK_OFF[b] + cw],
                        in_=o_t[b % NWK][:, 0:cw]).then_inc(s_out, 16)

                for b in range(NBK):
                    store(b)
                gp.wait_ge(s_out, 16 * NBK)

    nc.compile()
    return nc


# ---------------------------------------------------------------------------
# Fallback: previous all-f32r raw pipeline (measured ~105 us end-to-end).
# ---------------------------------------------------------------------------
LDR = 2048                 # raw-path input DMA batch (edges)
CHR = 512                  # raw-path matmul chunk


def _load_plan_raw():
    tail = [CHR, CHR, CHR, CHR // 2, CHR // 4, CHR // 4]
    loads = []
    pos = 0
    while pos < ES - sum(tail):
        loads.append((pos, LDR))
        pos += LDR
    for sz in tail:
        loads.append((pos, sz))
        pos += sz
    assert pos == ES, (pos, ES)
    return loads


def _build_bass_raw():
    import concourse.bacc as bacc
    from concourse import mybir

    f32 = mybir.dt.float32
    f32r = mybir.dt.float32r
    AF = mybir.ActivationFunctionType

    nc = bacc.Bacc("TRN2", target_bir_lowering=False, debug=False,
                   num_devices=N_CORES)

    xjT = nc.dram_tensor("xjT", [IN, ES], f32r, kind="ExternalInput")
    eijT = nc.dram_tensor("eijT", [IN, ES], f32r, kind="ExternalInput")
    xiT = nc.dram_tensor("xiT", [IN, ES], f32r, kind="ExternalInput")
    w1 = nc.dram_tensor("W1", [IN, F], f32r, kind="ExternalInput")
    w2 = nc.dram_tensor("W2", [IN, F], f32r, kind="ExternalInput")
    bv = nc.dram_tensor("b", [F, 1], f32, kind="ExternalInput")
    outT = nc.dram_tensor("outT", [F, ES], f32, kind="ExternalOutput")

    loads = _load_plan_raw()
    NB_ = len(loads)
    NIN_ = 5
    NWK_ = 3
    NEW_ = NWK_

    with ExitStack() as ctx:
        def mksem(name):
            return ctx.enter_context(nc.semaphore(name))

        s_xj = [mksem(f"s_xj{r}") for r in range(NIN_)]
        s_eij = [mksem(f"s_eij{r}") for r in range(NIN_)]
        s_xi = [mksem(f"s_xi{r}") for r in range(NIN_)]
        s_out = [mksem(f"s_out{r}") for r in range(NWK_)]
        s_mm = mksem("s_mm")
        s_red = mksem("s_red")
        s_rcp = mksem("s_rcp")
        s_psf = mksem("s_psf")
        s_exp = mksem("s_exp")
        s_mul = mksem("s_mul")
        s_const = mksem("s_const")

        in_xj = [ctx.enter_context(nc.sbuf_tensor(f"in_xj{r}", [IN, LDR], f32r))
                 for r in range(NIN_)]
        in_eij = [ctx.enter_context(nc.sbuf_tensor(f"in_eij{r}", [IN, LDR], f32r))
                  for r in range(NIN_)]
        in_xi = [ctx.enter_context(nc.sbuf_tensor(f"in_xi{r}", [IN, LDR], f32r))
                 for r in range(NIN_)]
        w_t = [ctx.enter_context(nc.sbuf_tensor(f"w{r}", [F, LDR], f32))
               for r in range(NWK_)]
        ew_t = [ctx.enter_context(nc.sbuf_tensor(f"ew{r}", [F, LDR], f32))
                for r in range(NEW_)]
        o_t = [ctx.enter_context(nc.sbuf_tensor(f"o{r}", [F, LDR], f32))
               for r in range(NWK_)]
        dn_t = ctx.enter_context(nc.sbuf_tensor("dn", [F, LDR // DEG], f32))
        rc_t = ctx.enter_context(nc.sbuf_tensor("rc", [F, LDR // DEG], f32))
        ps_t = [ctx.enter_context(nc.psum_tensor(f"ps{r}", [F, LDR], f32))
                for r in range(2)]
        w1_t = ctx.enter_context(nc.sbuf_tensor("w1s", [IN, F], f32r))
        w2_t = ctx.enter_context(nc.sbuf_tensor("w2s", [IN, F], f32r))
        b_t = ctx.enter_context(nc.sbuf_tensor("bs", [F, 1], f32))

        with nc.Block() as block:

            @block.sync
            def _(sp):
                for b, (pos, size) in enumerate(loads):
                    sl = slice(pos, pos + size)
                    if b >= NIN_:
                        sp.wait_ge(s_mm, b - (NIN_ - 1))
                    sp.dma_start(out=in_xj[b % NIN_][:, 0:size],
                                 in_=xjT[:, sl]).then_inc(s_xj[b % NIN_], 16)
                    if b % 2 == 0:
                        sp.dma_start(out=in_xi[b % NIN_][:, 0:size],
                                     in_=xiT[:, sl]).then_inc(s_xi[b % NIN_], 16)

            @block.scalar
            def _(act):
                def act_tail(bb):
                    bsz = loads[bb][1]
                    act.wait_ge(s_mm, bb + 1)
                    if bb >= NEW_:
                        act.wait_ge(s_mul, bb - (NEW_ - 1))
                    act.activation(
                        out=w_t[bb % NWK_][:, 0:bsz],
                        in_=ps_t[bb % 2][:, 0:bsz],
                        func=AF.Tanh, bias=b_t[:, 0:1],
                    ).then_inc(s_psf, 1)
                    if SAFE_INTRA:
                        act.wait_ge(s_psf, bb + 1)
                    act.activation(
                        out=ew_t[bb % NEW_][:, 0:bsz],
                        in_=w_t[bb % NWK_][:, 0:bsz],
                        func=AF.Exp,
                    ).then_inc(s_exp, 1)

                for b, (pos, size) in enumerate(loads):
                    sl = slice(pos, pos + size)
                    if b >= NIN_:
                        act.wait_ge(s_mm, b - (NIN_ - 1))
                    act.dma_start(out=in_eij[b % NIN_][:, 0:size],
                                  in_=eijT[:, sl]).then_inc(s_eij[b % NIN_], 16)
                    if b % 2 == 1:
                        act.dma_start(out=in_xi[b % NIN_][:, 0:size],
                                      in_=xiT[:, sl]).then_inc(s_xi[b % NIN_], 16)
                    if b >= 2:
                        bb = b - 2
                        if bb == 0:
                            act.wait_ge(s_const, 48)
                        act_tail(bb)
                for bb in (NB_ - 2, NB_ - 1):
                    act_tail(bb)

            @block.tensor
            def _(pe):
                pe.wait_ge(s_const, 48)
                for b, (pos, size) in enumerate(loads):
                    r = b % NIN_
                    n_use = b // NIN_ + 1
                    pe.wait_ge(s_xj[r], 16 * n_use)
                    pe.wait_ge(s_eij[r], 16 * n_use)
                    pe.wait_ge(s_xi[r], 16 * n_use)
                    if b >= 2:
                        pe.wait_ge(s_psf, b - 1)
                    ps = ps_t[b % 2]
                    nch = (size + CHR - 1) // CHR
                    for c in range(nch):
                        cw = min(CHR, size - c * CHR)
                        csl = slice(c * CHR, c * CHR + cw)
                        pe.matmul(ps[:, csl],
                                  w1_t[:], in_xj[b % NIN_][:, csl],
                                  start=True, stop=False)
                        pe.matmul(ps[:, csl],
                                  w1_t[:], in_eij[b % NIN_][:, csl],
                                  start=False, stop=False)
                        last = pe.matmul(ps[:, csl],
                                         w2_t[:], in_xi[b % NIN_][:, csl],
                                         start=False, stop=True)
                    last.then_inc(s_mm, 1)

            @block.vector
            def _(dve):
                for b, (pos, size) in enumerate(loads):
                    nseg = size // DEG
                    dve.wait_ge(s_exp, b + 1)
                    ew = ew_t[b % NEW_]
                    dve.reduce_sum(
                        out=dn_t[:, 0:nseg],
                        in_=ew[:, 0:size].rearrange("p (n d) -> p n d", d=DEG),
                        axis=mybir.AxisListType.X,
                    ).then_inc(s_red, 1)
                    if SAFE_INTRA:
                        dve.wait_ge(s_red, b + 1)
                    dve.reciprocal(
                        out=rc_t[:, 0:nseg], in_=dn_t[:, 0:nseg]
                    ).then_inc(s_rcp, 1)
                    if SAFE_INTRA:
                        dve.wait_ge(s_rcp, b + 1)
                    if b >= NWK_:
                        dve.wait_ge(s_out[b % NWK_], 16 * ((b - NWK_) // NWK_ + 1))
                    dve.tensor_mul(
                        out=o_t[b % NWK_][:, 0:size].rearrange(
                            "p (n d) -> p n d", d=DEG),
                        in0=ew[:, 0:size].rearrange("p (n d) -> p n d", d=DEG),
                        in1=rc_t[:, 0:nseg].unsqueeze(-1).broadcast_to(
                            [F, nseg, DEG]),
                    ).then_inc(s_mul, 1)

            @block.gpsimd
            def _(gp):
                gp.dma_start(out=w1_t[:], in_=w1[:]).then_inc(s_const, 16)
                gp.dma_start(out=w2_t[:], in_=w2[:]).then_inc(s_const, 16)
                gp.dma_start(out=b_t[:], in_=bv[:]).then_inc(s_const, 16)
                for b, (pos, size) in enumerate(loads):
                    sl = slice(pos, pos + size)
                    gp.wait_ge(s_mul, b + 1)
                    gp.dma_start(
                        out=outT[:, sl],
                        in_=o_t[b % NWK_][:, 0:size],
                    ).then_inc(s_out[b % NWK_], 16)
                for r in range(NWK_):
                    n_r = len(range(r, NB_, NWK_))
                    gp.wait_ge(s_out[r], 16 * n_r)

    nc.compile()
    return nc


def _get_compiled(mode):
    if mode not in _COMPILED:
        _COMPILED[mode] = _build_fp8() if mode == "fp8" else _build_bass_raw()
    return _COMPILED[mode]


def _prep_inputs_fp8(x_i, x_j, e_ij, W, b):
    import ml_dtypes

    F8 = ml_dtypes.float8_e3m4
    BF16 = ml_dtypes.bfloat16

    W = np.asarray(W, dtype=np.float32)
    W1 = np.ascontiguousarray(W[:IN]).astype(BF16)
    W2 = np.ascontiguousarray(W[IN:]).astype(BF16)
    bias = np.asarray(b, dtype=np.float32).reshape(F)
    btile = np.zeros((IN, 1), np.float32)
    for cc in range(NCT):
        btile[32 * cc:32 * cc + F, 0] = bias

    in_maps = []
    for c in range(N_CORES):
        sl = slice(c * ES, (c + 1) * ES)
        xjT = np.ascontiguousarray(np.asarray(x_j[sl], np.float32).T)
        eijT = np.ascontiguousarray(np.asarray(e_ij[sl], np.float32).T)
        xiT = np.ascontiguousarray(np.asarray(x_i[sl], np.float32).T)
        xj8 = xjT.astype(F8)
        # error feedback: fold xj's quantization error into eij before its
        # quantization, so q = xj + eij carries a single quantization error
        eij8 = (eijT + (xjT - xj8.astype(np.float32))).astype(F8)
        xi8 = xiT.astype(F8)
        # pack per load: contiguous [IN, 3*size] block b = [xj | eij | xi]
        blocks = []
        for b in range(NBK):
            pos, size = BANK_POS[b], BANK_SIZES[b]
            blk = np.concatenate([xj8[:, pos:pos + size],
                                  eij8[:, pos:pos + size],
                                  xi8[:, pos:pos + size]], axis=1)
            blocks.append(np.ascontiguousarray(blk).reshape(-1))
        pk = np.concatenate(blocks)
        in_maps.append({
            "pk": pk,
            "W1": W1,
            "W2": W2,
            "b": btile,
        })
    return in_maps


def _gather_fp8(res):
    out = np.empty((E, F), dtype=np.float32)
    for c in range(N_CORES):
        o2 = np.asarray(res.results[c]["out2"]).astype(np.float32)
        oc = out[c * ES:(c + 1) * ES]
        for b in range(NBK):
            pos, cw = BANK_POS[b], BANK_CHW[b]
            blk = o2.reshape(NCT, 32, OUT_COLS)[:, :F,
                                                BANK_OFF[b]:BANK_OFF[b] + cw]
            oc[pos:pos + NCT * cw] = blk.transpose(0, 2, 1).reshape(-1, F)
    return out


def _prep_inputs_raw(x_i, x_j, e_ij, W, b):
    W = np.ascontiguousarray(np.asarray(W, dtype=np.float32))
    bias = np.asarray(b, dtype=np.float32).reshape(F, 1)
    W1 = np.ascontiguousarray(W[:IN])
    W2 = np.ascontiguousarray(W[IN:])
    in_maps = []
    for c in range(N_CORES):
        sl = slice(c * ES, (c + 1) * ES)
        in_maps.append({
            "xjT": np.ascontiguousarray(np.asarray(x_j[sl]).T),
            "eijT": np.ascontiguousarray(np.asarray(e_ij[sl]).T),
            "xiT": np.ascontiguousarray(np.asarray(x_i[sl]).T),
            "W1": W1,
            "W2": W2,
            "b": bias,
        })
    return in_maps


def _gather_raw(res):
    out = np.empty((E, F), dtype=np.float32)
    for c in range(N_CORES):
        out[c * ES:(c + 1) * ES] = np.asarray(res.results[c]["outT"]).T
    return out


def _run_device(x_i, x_j, e_ij, W, b, trace=False, tmpdir=None,
                trace_cores=None, mode="fp8"):
    from concourse.bass_utils import run_bass_kernel_spmd

    nc = _get_compiled(mode)
    if mode == "fp8":
        in_maps = _prep_inputs_fp8(x_i, x_j, e_ij, W, b)
    else:
        in_maps = _prep_inputs_raw(x_i, x_j, e_ij, W, b)

    kwargs = {}
    if trace:
        kwargs.update(trace=True,
                      trace_cores=(trace_cores if trace_cores is not None
                                   else list(range(N_CORES))),
                      tmpdir=tmpdir)
    res = run_bass_kernel_spmd(nc, in_maps, core_ids=list(range(N_CORES)),
                               **kwargs)

    out = _gather_fp8(res) if mode == "fp8" else _gather_raw(res)
    return out, res


def _numpy_fallback(x_i, x_j, e_ij, adj, e_row, W, b):
    """Correct for arbitrary e_row (matches the reference semantics)."""
    x_i = np.asarray(x_i, np.float32)
    x_j = np.asarray(x_j, np.float32)
    e_ij = np.asarray(e_ij, np.float32)
    W = np.asarray(W, np.float32)
    b = np.asarray(b, np.float32)
    e_row = np.asarray(e_row).astype(np.int64)
    n = np.asarray(adj).shape[0]
    q = x_j + e_ij
    z = q @ W[:q.shape[1]] + x_i @ W[q.shape[1]:] + b
    w = np.tanh(z)
    m = np.full((n, w.shape[1]), -9e15, np.float32)
    np.maximum.at(m, e_row, w)
    ew = np.exp(w - m[e_row])
    denom = np.zeros((n, w.shape[1]), np.float32)
    np.add.at(denom, e_row, ew)
    return (ew / denom[e_row]).astype(np.float32)


def _is_fast_path(x_i, x_j, e_ij, adj, e_row, W, b):
    try:
        if np.asarray(x_i).shape != (E, IN):
            return False
        if np.asarray(x_j).shape != (E, IN):
            return False
        if np.asarray(e_ij).shape != (E, IN):
            return False
        if np.asarray(W).shape != (2 * IN, F):
            return False
        if np.asarray(b).reshape(-1).shape != (F,):
            return False
        if np.asarray(adj).shape[0] != N_NODES:
            return False
        er = np.asarray(e_row).reshape(-1)
        if er.shape != (E,):
            return False
        expected = np.repeat(np.arange(N_NODES, dtype=np.int64), DEG)
        return bool(np.array_equal(er.astype(np.int64), expected))
    except Exception:
        return False


def kernel(x_i, x_j, e_ij, adj, e_row, e_col, W, b, **_unused):
    if _is_fast_path(x_i, x_j, e_ij, adj, e_row, W, b):
        for mode in ("fp8", "raw"):
            try:
                out, _ = _run_device(x_i, x_j, e_ij, W, b, mode=mode)
                return out
            except Exception as e:  # fail safe: correct > fast
                print(f"kernel: device path '{mode}' failed "
                      f"({type(e).__name__}: {e}); trying next",
                      file=sys.stderr)
    return _numpy_fallback(x_i, x_j, e_ij, adj, e_row, W, b)
